# revision 78
# baseline (speedup 1.0000x reference)
"""Trainium2 Bass kernel for nn_AttentionLayer (attention pooling).

Reference math (per batch row b):
    u   = tanh(x[b] @ W + b_vec)        # [T, M]
    s   = u @ us                        # [T]
    a   = softmax(s) * mask / sum       # [T]  (mask is all ones per spec)
    out = a @ x[b]                      # [D]

Strategy: data-parallel over batch, B=32 rows -> 4 rows per NeuronCore on
8 cores.  Per core the kernel is HBM-DMA-bound (~94us to stream x at the
~358 GB/s per-core share of stack bandwidth), so the emission is built
around keeping that stream and the PE continuously busy:
  - x streams in as 1MB half-quarter tiles on the sync HWDGE queue with a
    ~6-tile lookahead; W/b/us and y outputs use the ACT HWDGE queue so
    they never stall the x stream;
  - fp32->bf16 casts per half on DVE; PE transposes x 128x128 blocks via
    identity matmuls (~56ns/tile back-to-back);
  - u^T = tanh(W^T x^T + bias) accumulates in PSUM, tanh fused on ScalarE;
  - scores via matmul(lhsT=u^T chunk, rhs=us); exp on ScalarE with
    accum_out row sums (no max subtraction needed: |s| <= ~5);
  - out = (e^T x) / sum(e): two PE column-groups (q0/q64) compute the two
    d-halves concurrently into separate PSUM banks.
The PE warm-up keeps the HAM clock gate at K=8/8 through the first real
work, and extra warm-up matmuls are interleaved into quarter 0's
transpose-only stretch so the activity window never lapses.
"""
import numpy as np

import concourse.bacc as bacc
import concourse.mybir as mybir
from concourse.tile import TileContext
from concourse.masks import make_identity
from concourse.bass_utils import run_bass_kernel_spmd

F32 = mybir.dt.float32
BF16 = mybir.dt.bfloat16

B, T, D, M = 32, 2048, 1024, 128
NCORES = 8
B_SH = B // NCORES   # 4 batch rows per core
P = 128
NT = T // P          # 16 t-tiles per row
NCD = D // P         # 8 d-chunks
QT = 4               # t-tiles per quarter-row
NQ = NT // QT        # 4 quarters per row
NH = B_SH * NT // 2  # 32 half-quarter DMA tiles ([P, 2, D] each)
WARMUP = 56
WARMUP_FILL = 16
LOOKAHEAD = 6


def _build_nc():
    nc = bacc.Bacc("TRN2", target_bir_lowering=False, debug=False,
                   num_devices=NCORES)
    x = nc.declare_dram_parameter("x", [B_SH, T, D], F32, isOutput=False)
    # W arrives host-rearranged to lhsT layout: W_r[p, c, m] = W[128c+p, m]
    W = nc.declare_dram_parameter("W", [P, NCD, M], F32, isOutput=False)
    b = nc.declare_dram_parameter("b", [1, M], F32, isOutput=False)
    us = nc.declare_dram_parameter("us", [1, M], F32, isOutput=False)
    y = nc.declare_dram_parameter("y", [B_SH, D], F32, isOutput=True)

    with TileContext(nc) as tc:
        with (
            tc.tile_pool(name="singles", bufs=1) as singles,
            tc.tile_pool(name="stage", bufs=4) as stage,
            tc.tile_pool(name="xb", bufs=2) as xb_pool,
            tc.tile_pool(name="xt", bufs=3) as xt_pool,
            tc.tile_pool(name="u", bufs=2) as u_pool,
            tc.tile_pool(name="et", bufs=4) as et_pool,
            tc.tile_pool(name="outs", bufs=2) as out_pool,
            tc.tile_pool(name="tp_ps", bufs=3, space="PSUM") as tp_psum,
            tc.tile_pool(name="u_ps", bufs=2, space="PSUM") as u_psum,
            tc.tile_pool(name="s_ps", bufs=1, space="PSUM") as s_psum,
            tc.tile_pool(name="o_ps", bufs=2, space="PSUM") as o_psum,
        ):
            # constants go on the ACT HWDGE queue FIRST so they beat the x
            # flood; W is host-rearranged so this is a contiguous 4KB/partition
            # load, b/us are small
            w_f32 = singles.tile([P, NCD, M], F32)
            nc.scalar.dma_start(out=w_f32, in_=W[:, :, :])
            b_row = singles.tile([1, M], F32)
            nc.scalar.dma_start(out=b_row, in_=b[:, :])
            us_row = singles.tile([1, M], F32)
            nc.scalar.dma_start(out=us_row, in_=us[:, :])

            # x streams on the sync HWDGE queue: quarter tiles, each filled
            # by two 1MB half DMAs so the stream stays fine-grained
            stage_tiles = {}
            emitted = [0]

            def emit_stage():
                h = emitted[0]
                idx, half = divmod(h, 2)
                r, q = divmod(idx, NQ)
                if half == 0:
                    st_new = stage.tile([P, QT, D], F32, tag="stage",
                                        name=f"stage_{idx}")
                    stage_tiles[idx] = st_new
                st = stage_tiles[idx]
                nc.sync.dma_start(
                    out=st[:, 2 * half:2 * half + 2, :],
                    in_=x[r].rearrange("(n p) d -> p n d", p=P)[
                        :, q * QT + 2 * half:q * QT + 2 * half + 2, :],
                )
                emitted[0] += 1

            emit_stage()  # first x tile right behind the constants

            # warm-up constant on DVE only (no gpsimd library wait)
            wones = singles.tile([P, P], BF16)
            nc.vector.memset(wones, 1.0)
            ones_f32 = singles.tile([P, 1], F32)
            nc.vector.memset(ones_f32, 1.0)
            one_f32 = singles.tile([1, 1], F32)
            nc.vector.memset(one_f32, 1.0)

            ident = singles.tile([P, P], BF16)
            make_identity(nc, ident)

            w_bf = singles.tile([P, NCD, M], BF16)
            nc.vector.tensor_copy(out=w_bf, in_=w_f32)
            b_sb = singles.tile([P, 1], F32)
            us_bf = singles.tile([P, 1], BF16)

            while emitted[0] < LOOKAHEAD:
                emit_stage()

            # PE warm-up: lift HAM to K=8/8 while the first DMAs stream and
            # hand off to the first transposes without an idle window
            warm = u_psum.tile([P, QT * P], F32, tag="up")
            for i in range(WARMUP):
                nc.tensor.matmul(warm[:, :P], wones, wones, start=True, stop=True)

            # transpose b/us rows to per-partition layout on the PE; their
            # single-descriptor DMAs land fast so nothing blocks on them
            bc_ps = s_psum.tile([P, 2], F32, tag="s")
            nc.tensor.matmul(bc_ps[:, 0:1], b_row, one_f32, start=True, stop=True)
            nc.tensor.matmul(bc_ps[:, 1:2], us_row, one_f32, start=True, stop=True)

            # software pipeline over all quarters; c1/c2 op lists for
            # quarter idx drain interleaved with transposes of idx+1/idx+2
            rowstate = {}
            compute1 = {}
            compute1b = {}
            compute2 = {}

            def drain(ops, k):
                for _ in range(k):
                    if ops:
                        ops.pop(0)()

            def warm_fill(n=WARMUP_FILL):
                ops = []
                for i in range(n):
                    def f():
                        nc.tensor.matmul(warm[:, :P], wones, wones,
                                         start=True, stop=True)
                    ops.append(f)
                return ops

            # fillers bridge the HAM activity window from warm-up through the
            # first two quarters' sparse stretch; only idx 0/1 may write the
            # warm tile (its PSUM ring slot is reused from quarter 1 on)
            compute1[-1] = warm_fill(32)
            compute2[-2] = warm_fill(16)
            compute2[-1] = warm_fill(12)

            for idx in range(B_SH * NQ):
                r, q = divmod(idx, NQ)
                if q == 0:
                    rowstate[r] = dict(
                        x_bf=xb_pool.tile([P, NT, D], BF16, tag="xb", name=f"x_bf_{r}"),
                        u_sb=u_pool.tile([P, T], BF16, tag="u", name=f"u_sb_{r}"),
                        rs=out_pool.tile([P, NQ], F32, tag="rs", name=f"rs_{r}"),
                        opa=o_psum.tile([P, 512], F32, tag="o", name=f"opa_{r}"),
                        opb=o_psum.tile([P, 512], F32, tag="o", name=f"opb_{r}"),
                        o_sb=out_pool.tile([1, D], F32, tag="o_sb", name=f"o_sb_{r}"),
                    )
                rs_ = rowstate[r]
                x_bf, u_sb = rs_["x_bf"], rs_["u_sb"]

                # cast for this quarter (DVE), with the DMA stream kept
                # LOOKAHEAD halves ahead
                while emitted[0] < min(idx * 2 + LOOKAHEAD, NH):
                    emit_stage()
                st = stage_tiles.pop(idx)
                if idx == B_SH * NQ - 1:
                    # last quarter: per-half casts so the post-stream chain
                    # starts as soon as the second-to-last DMA lands
                    nc.vector.tensor_copy(
                        out=x_bf[:, q * QT:q * QT + 2, :], in_=st[:, 0:2, :])
                    nc.vector.tensor_copy(
                        out=x_bf[:, q * QT + 2:q * QT + 4, :], in_=st[:, 2:4, :])
                else:
                    nc.vector.tensor_copy(
                        out=x_bf[:, q * QT:(q + 1) * QT, :], in_=st)
                if idx == 0:
                    nc.vector.tensor_copy(out=b_sb, in_=bc_ps[:, 0:1])
                    nc.vector.tensor_copy(out=us_bf, in_=bc_ps[:, 1:2])

                c1 = compute1.pop(idx - 1, [])
                c1b = compute1b.pop(idx - 1, [])
                c2 = compute2.pop(idx - 2, [])

                # quarter layout chosen to minimize PE context switches
                # (~96ns each): one xW block and one c2 block up front (they
                # also cover the cast latency), then all transposes as one
                # batch, then scores+exp whose tanh is long since done
                drain(c1, len(c1))
                drain(c2, len(c2))
                xt = xt_pool.tile([P, QT, NCD, P], BF16, tag="xt")
                for j in range(QT):
                    t_idx = q * QT + j
                    for g in range(2):
                        tp = tp_psum.tile([P, 4 * P], BF16, tag="tp")
                        for cc in range(4):
                            c = g * 4 + cc
                            nc.tensor.transpose(
                                tp[:, cc * P:(cc + 1) * P],
                                x_bf[:, t_idx, c * P:(c + 1) * P],
                                ident,
                            )
                        dst = xt[:, j, g * 4:(g + 1) * 4, :]
                        src = tp.rearrange("p (c t) -> p c t", c=4)
                        if g == 0:
                            nc.scalar.copy(out=dst, in_=src)
                        else:
                            nc.vector.tensor_copy(out=dst, in_=src)
                drain(c1b, len(c1b))

                def make_c1(r=r, q=q, xt=xt, u_sb=u_sb, rs_=rs_,
                            split=(idx == B_SH * NQ - 1)):
                    ops = []
                    up = u_psum.tile([P, QT * P], F32, tag="up")

                    def mk_p1(c, j0=0, j1=QT):
                        def f():
                            nc.tensor.matmul(
                                up[:, j0 * P:j1 * P],
                                w_bf[:, c, :], xt[:, j0:j1, c, :],
                                start=(c == 0), stop=(c == NCD - 1),
                            )
                        return f
                    if split:
                        # last quarter: two column halves so the first half's
                        # matmuls start after only two transpose-copies land
                        for c in range(NCD):
                            ops.append(mk_p1(c, 0, 2))
                        for c in range(NCD):
                            ops.append(mk_p1(c, 2, QT))
                    else:
                        for c in range(NCD):
                            ops.append(mk_p1(c))

                    def tanh_op():
                        nc.scalar.activation(
                            out=u_sb[:, q * QT * P:(q + 1) * QT * P], in_=up,
                            func=mybir.ActivationFunctionType.Tanh,
                            bias=b_sb, scale=1.0,
                        )
                    ops.append(tanh_op)
                    return ops

                compute1[idx] = make_c1()

                def make_c1b(r=r, q=q, u_sb=u_sb, rs_=rs_):
                    ops = []
                    sp = s_psum.tile([P, QT], F32, tag="s")

                    def mk_st(j):
                        def f():
                            t_idx = q * QT + j
                            nc.tensor.matmul(
                                sp[:, j:j + 1],
                                u_sb[:, t_idx * P:(t_idx + 1) * P],
                                us_bf, start=True, stop=True,
                            )
                        return f
                    for j in range(QT):
                        ops.append(mk_st(j))

                    etq = et_pool.tile([P, QT], BF16, tag="et")
                    rs_[f"et{q}"] = etq

                    def exp_op():
                        nc.scalar.activation(
                            out=etq, in_=sp,
                            func=mybir.ActivationFunctionType.Exp,
                            accum_out=rs_["rs"][:, q:q + 1],
                        )
                    ops.append(exp_op)
                    return ops

                compute1b[idx] = make_c1b()

                def make_c2(r=r, q=q, x_bf=x_bf, rs_=rs_):
                    # each list entry emits one t-tile's pair of column-group
                    # matmuls back-to-back so they co-run on the PE
                    ops = []

                    def mk_p2(j):
                        def f():
                            t_idx = q * QT + j
                            for g in range(2):
                                op_t = rs_["opa"] if g == 0 else rs_["opb"]
                                kwargs = {}
                                if g == 1:
                                    kwargs["tile_position"] = (0, 64)
                                nc.tensor.matmul(
                                    op_t[64 * g:64 * g + 1, :],
                                    rs_[f"et{q}"][:, j:j + 1],
                                    x_bf[:, t_idx, g * 512:(g + 1) * 512],
                                    start=(q == 0 and j == 0),
                                    stop=(q == NQ - 1 and j == QT - 1),
                                    **kwargs,
                                )
                        return f
                    for j in range(QT):
                        ops.append(mk_p2(j))

                    if q == NQ - 1:
                        def finish():
                            dnp = s_psum.tile([1, NQ], F32, tag="s")
                            nc.tensor.matmul(dnp, ones_f32, rs_["rs"],
                                             start=True, stop=True)
                            dsum = out_pool.tile([1, 1], F32, tag="dsum")
                            nc.vector.reduce_sum(out=dsum, in_=dnp,
                                                 axis=mybir.AxisListType.X)
                            inv = out_pool.tile([1, 1], F32, tag="inv")
                            nc.vector.reciprocal(out=inv, in_=dsum)
                            o_sb = rs_["o_sb"]
                            nc.vector.tensor_scalar_mul(
                                o_sb[:, 0:512], rs_["opa"][0:1, :], inv)
                            nc.vector.tensor_scalar_mul(
                                o_sb[:, 512:1024], rs_["opb"][64:65, :], inv)
                            nc.scalar.dma_start(out=y[r:r + 1, :], in_=o_sb)
                        ops.append(finish)
                    return ops

                compute2[idx] = make_c2()

            # tail: drain ready work (c2 of the second-to-last quarter)
            # first so the PE chews it while the last quarter's copies land
            for idx in sorted(set(compute1) | set(compute1b) | set(compute2)):
                for f in compute2.pop(idx - 1, []):
                    f()
                for f in compute1.pop(idx, []):
                    f()
                for f in compute1b.pop(idx, []):
                    f()
                for f in compute2.pop(idx, []):
                    f()

    nc.compile()
    return nc


_NC_CACHE = []


def _numpy_reference(x, W, b, us, mask):
    m = mask.astype(x.dtype)
    u = np.tanh(np.einsum('btd,dm->btm', x, W) + b)
    utu = np.einsum('btm,mo->bto', u, us)[..., 0]
    e = np.exp(utu - utu.max(axis=-1, keepdims=True))
    e = m * e
    a = e / e.sum(axis=-1, keepdims=True)
    return np.einsum('bt,btd->bd', a, x).astype(np.float32)


def make_in_maps(x, W, b, us):
    """Per-core input dicts; W goes in host-rearranged lhsT layout."""
    x = np.ascontiguousarray(np.asarray(x, dtype=np.float32))
    W = np.ascontiguousarray(np.asarray(W, dtype=np.float32))
    b = np.ascontiguousarray(np.asarray(b, dtype=np.float32))
    us = np.ascontiguousarray(np.asarray(us, dtype=np.float32))
    W_r = np.ascontiguousarray(W.reshape(NCD, P, M).transpose(1, 0, 2))
    b_r = np.ascontiguousarray(b.reshape(1, M))
    us_r = np.ascontiguousarray(us.reshape(M, 1).T)
    return [{
        "x": np.ascontiguousarray(x[i * B_SH:(i + 1) * B_SH]),
        "W": W_r, "b": b_r, "us": us_r,
    } for i in range(NCORES)]


def kernel(x, W, b, us, mask):
    x = np.ascontiguousarray(np.asarray(x, dtype=np.float32))
    W = np.ascontiguousarray(np.asarray(W, dtype=np.float32))
    b = np.ascontiguousarray(np.asarray(b, dtype=np.float32))
    us = np.ascontiguousarray(np.asarray(us, dtype=np.float32))
    mask = np.asarray(mask)

    if not bool(mask.all()):
        # spec guarantees an all-ones mask; fall back to exact numpy
        # reference if that ever changes
        return _numpy_reference(x, W, b, us, mask)

    if not _NC_CACHE:
        _NC_CACHE.append(_build_nc())
    nc = _NC_CACHE[0]

    in_maps = make_in_maps(x, W, b, us)
    res = run_bass_kernel_spmd(nc, in_maps, core_ids=list(range(NCORES)),
                               trace=False)
    return np.concatenate([res.results[i]["y"] for i in range(NCORES)], axis=0)


# revision 83
# speedup vs baseline: 1.0215x; 1.0215x over previous
"""Trainium2 Bass kernel for nn_AttentionLayer (attention pooling).

Reference math (per batch row b):
    u   = tanh(x[b] @ W + b_vec)        # [T, M]
    s   = u @ us                        # [T]
    a   = softmax(s) * mask / sum       # [T]  (mask is all ones per spec)
    out = a @ x[b]                      # [D]

Strategy: data-parallel over batch, B=32 rows -> 4 rows per NeuronCore on
8 cores.  Per core the kernel is HBM-DMA-bound (~94us to stream x at the
~358 GB/s per-core share of stack bandwidth), so the emission is built
around keeping that stream and the PE continuously busy:
  - x streams in as 1MB half-quarter tiles on the sync HWDGE queue with a
    ~6-tile lookahead; W/b/us and y outputs use the ACT HWDGE queue so
    they never stall the x stream;
  - fp32->bf16 casts per half on DVE; PE transposes x 128x128 blocks via
    identity matmuls (~56ns/tile back-to-back);
  - u^T = tanh(W^T x^T + bias) accumulates in PSUM, tanh fused on ScalarE;
  - scores via matmul(lhsT=u^T chunk, rhs=us); exp on ScalarE with
    accum_out row sums (no max subtraction needed: |s| <= ~5);
  - out = (e^T x) / sum(e): two PE column-groups (q0/q64) compute the two
    d-halves concurrently into separate PSUM banks.
The PE warm-up keeps the HAM clock gate at K=8/8 through the first real
work, and extra warm-up matmuls are interleaved into quarter 0's
transpose-only stretch so the activity window never lapses.
"""
import numpy as np

import concourse.bacc as bacc
import concourse.mybir as mybir
from concourse.tile import TileContext
from concourse.masks import make_identity
from concourse.bass_utils import run_bass_kernel_spmd

F32 = mybir.dt.float32
BF16 = mybir.dt.bfloat16

B, T, D, M = 32, 2048, 1024, 128
NCORES = 8
B_SH = B // NCORES   # 4 batch rows per core
P = 128
NT = T // P          # 16 t-tiles per row
NCD = D // P         # 8 d-chunks
QT = 4               # t-tiles per quarter-row
NQ = NT // QT        # 4 quarters per row
NH = B_SH * NT // 2  # 32 half-quarter DMA tiles ([P, 2, D] each)
WARMUP = 56
WARMUP_FILL = 16
LOOKAHEAD = 6


def _build_nc():
    nc = bacc.Bacc("TRN2", target_bir_lowering=False, debug=False,
                   num_devices=NCORES)
    x = nc.declare_dram_parameter("x", [B_SH, T, D], F32, isOutput=False)
    # W arrives host-rearranged to lhsT layout: W_r[p, c, m] = W[128c+p, m]
    W = nc.declare_dram_parameter("W", [P, NCD, M], F32, isOutput=False)
    b = nc.declare_dram_parameter("b", [1, M], F32, isOutput=False)
    us = nc.declare_dram_parameter("us", [1, M], F32, isOutput=False)
    y = nc.declare_dram_parameter("y", [B_SH, D], F32, isOutput=True)

    with TileContext(nc) as tc:
        with (
            tc.tile_pool(name="singles", bufs=1) as singles,
            tc.tile_pool(name="stage", bufs=4) as stage,
            tc.tile_pool(name="xb", bufs=2) as xb_pool,
            tc.tile_pool(name="xt", bufs=3) as xt_pool,
            tc.tile_pool(name="u", bufs=2) as u_pool,
            tc.tile_pool(name="et", bufs=4) as et_pool,
            tc.tile_pool(name="outs", bufs=2) as out_pool,
            tc.tile_pool(name="tp_ps", bufs=3, space="PSUM") as tp_psum,
            tc.tile_pool(name="u_ps", bufs=2, space="PSUM") as u_psum,
            tc.tile_pool(name="s_ps", bufs=1, space="PSUM") as s_psum,
            tc.tile_pool(name="o_ps", bufs=2, space="PSUM") as o_psum,
        ):
            # constants go on the ACT HWDGE queue FIRST so they beat the x
            # flood; W is host-rearranged so this is a contiguous 4KB/partition
            # load, b/us are small
            w_f32 = singles.tile([P, NCD, M], F32)
            nc.scalar.dma_start(out=w_f32, in_=W[:, :, :])
            b_row = singles.tile([1, M], F32)
            nc.scalar.dma_start(out=b_row, in_=b[:, :])
            us_row = singles.tile([1, M], F32)
            nc.scalar.dma_start(out=us_row, in_=us[:, :])

            # x streams on the sync HWDGE queue: quarter tiles, each filled
            # by two 1MB half DMAs so the stream stays fine-grained
            stage_tiles = {}
            emitted = [0]

            def emit_stage():
                h = emitted[0]
                idx, half = divmod(h, 2)
                r, q = divmod(idx, NQ)
                if half == 0:
                    st_new = stage.tile([P, QT, D], F32, tag="stage",
                                        name=f"stage_{idx}")
                    stage_tiles[idx] = st_new
                st = stage_tiles[idx]
                nc.sync.dma_start(
                    out=st[:, 2 * half:2 * half + 2, :],
                    in_=x[r].rearrange("(n p) d -> p n d", p=P)[
                        :, q * QT + 2 * half:q * QT + 2 * half + 2, :],
                )
                emitted[0] += 1

            emit_stage()  # first x tile right behind the constants

            # warm-up constant on DVE only (no gpsimd library wait)
            wones = singles.tile([P, P], BF16)
            nc.vector.memset(wones, 1.0)
            ones_f32 = singles.tile([P, 1], F32)
            nc.vector.memset(ones_f32, 1.0)
            one_f32 = singles.tile([1, 1], F32)
            nc.vector.memset(one_f32, 1.0)

            ident = singles.tile([P, P], BF16)
            make_identity(nc, ident)

            w_bf = singles.tile([P, NCD, M], BF16)
            nc.vector.tensor_copy(out=w_bf, in_=w_f32)
            b_sb = singles.tile([P, 1], F32)
            us_bf = singles.tile([P, 1], BF16)

            while emitted[0] < LOOKAHEAD:
                emit_stage()

            # PE warm-up: lift HAM to K=8/8 while the first DMAs stream and
            # hand off to the first transposes without an idle window
            warm = u_psum.tile([P, QT * P], F32, tag="up")
            for i in range(WARMUP):
                nc.tensor.matmul(warm[:, :P], wones, wones, start=True, stop=True)

            # transpose b/us rows to per-partition layout on the PE; their
            # single-descriptor DMAs land fast so nothing blocks on them
            bc_ps = s_psum.tile([P, 2], F32, tag="s")
            nc.tensor.matmul(bc_ps[:, 0:1], b_row, one_f32, start=True, stop=True)
            nc.tensor.matmul(bc_ps[:, 1:2], us_row, one_f32, start=True, stop=True)

            # software pipeline over all quarters; c1/c2 op lists for
            # quarter idx drain interleaved with transposes of idx+1/idx+2
            rowstate = {}
            compute1 = {}
            compute1b = {}
            compute2 = {}

            def drain(ops, k):
                for _ in range(k):
                    if ops:
                        ops.pop(0)()

            def warm_fill(n=WARMUP_FILL):
                ops = []
                for i in range(n):
                    def f():
                        nc.tensor.matmul(warm[:, :P], wones, wones,
                                         start=True, stop=True)
                    ops.append(f)
                return ops

            # fillers bridge the HAM activity window from warm-up through the
            # first two quarters' sparse stretch; only idx 0/1 may write the
            # warm tile (its PSUM ring slot is reused from quarter 1 on)
            compute1[-1] = warm_fill(32)
            compute2[-2] = warm_fill(16)
            compute2[-1] = warm_fill(12)

            for idx in range(B_SH * NQ):
                r, q = divmod(idx, NQ)
                if q == 0:
                    rowstate[r] = dict(
                        x_bf=xb_pool.tile([P, NT, D], BF16, tag="xb", name=f"x_bf_{r}"),
                        u_sb=u_pool.tile([P, T], BF16, tag="u", name=f"u_sb_{r}"),
                        rs=out_pool.tile([P, NQ + 1], F32, tag="rs", name=f"rs_{r}"),
                        opa=o_psum.tile([P, 512], F32, tag="o", name=f"opa_{r}"),
                        opb=o_psum.tile([P, 512], F32, tag="o", name=f"opb_{r}"),
                        o_sb=out_pool.tile([1, D], F32, tag="o_sb", name=f"o_sb_{r}"),
                    )
                rs_ = rowstate[r]
                x_bf, u_sb = rs_["x_bf"], rs_["u_sb"]

                # cast for this quarter (DVE), with the DMA stream kept
                # LOOKAHEAD halves ahead
                while emitted[0] < min(idx * 2 + LOOKAHEAD, NH):
                    emit_stage()
                st = stage_tiles.pop(idx)
                if idx == B_SH * NQ - 1:
                    # last quarter: per-half casts so the post-stream chain
                    # starts as soon as the second-to-last DMA lands
                    nc.vector.tensor_copy(
                        out=x_bf[:, q * QT:q * QT + 2, :], in_=st[:, 0:2, :])
                    nc.vector.tensor_copy(
                        out=x_bf[:, q * QT + 2:q * QT + 4, :], in_=st[:, 2:4, :])
                else:
                    nc.vector.tensor_copy(
                        out=x_bf[:, q * QT:(q + 1) * QT, :], in_=st)
                if idx == 0:
                    nc.vector.tensor_copy(out=b_sb, in_=bc_ps[:, 0:1])
                    nc.vector.tensor_copy(out=us_bf, in_=bc_ps[:, 1:2])

                c1 = compute1.pop(idx - 1, [])
                c1b = compute1b.pop(idx - 1, [])
                c2 = compute2.pop(idx - 2, [])

                # quarter layout chosen to minimize PE context switches
                # (~96ns each): one xW block and one c2 block up front (they
                # also cover the cast latency), then all transposes as one
                # batch, then scores+exp whose tanh is long since done
                drain(c1, len(c1))
                drain(c2, len(c2))
                xt = xt_pool.tile([P, QT, NCD, P], BF16, tag="xt")
                for j in range(QT):
                    t_idx = q * QT + j
                    for g in range(2):
                        tp = tp_psum.tile([P, 4 * P], BF16, tag="tp")
                        for cc in range(4):
                            c = g * 4 + cc
                            nc.tensor.transpose(
                                tp[:, cc * P:(cc + 1) * P],
                                x_bf[:, t_idx, c * P:(c + 1) * P],
                                ident,
                            )
                        dst = xt[:, j, g * 4:(g + 1) * 4, :]
                        src = tp.rearrange("p (c t) -> p c t", c=4)
                        if g == 0:
                            nc.scalar.copy(out=dst, in_=src)
                        else:
                            nc.vector.tensor_copy(out=dst, in_=src)
                drain(c1b, len(c1b))

                def make_c1(r=r, q=q, xt=xt, u_sb=u_sb, rs_=rs_,
                            split=(idx == B_SH * NQ - 1)):
                    ops = []
                    up = u_psum.tile([P, QT * P], F32, tag="up")

                    def mk_p1(c, j0=0, j1=QT):
                        def f():
                            nc.tensor.matmul(
                                up[:, j0 * P:j1 * P],
                                w_bf[:, c, :], xt[:, j0:j1, c, :],
                                start=(c == 0), stop=(c == NCD - 1),
                            )
                        return f
                    def tanh_half(h):
                        def f():
                            lo = q * QT * P + h * 2 * P
                            nc.scalar.activation(
                                out=u_sb[:, lo:lo + 2 * P],
                                in_=up[:, h * 2 * P:(h + 1) * 2 * P],
                                func=mybir.ActivationFunctionType.Tanh,
                                bias=b_sb, scale=1.0,
                            )
                        return f

                    if split:
                        # last quarter: two independent half-pipelines so the
                        # first half's tanh runs while the second half's
                        # matmuls still stream
                        for c in range(NCD):
                            ops.append(mk_p1(c, 0, 2))
                        ops.append(tanh_half(0))
                        for c in range(NCD):
                            ops.append(mk_p1(c, 2, QT))
                        ops.append(tanh_half(1))
                    else:
                        for c in range(NCD):
                            ops.append(mk_p1(c))

                        def tanh_op():
                            nc.scalar.activation(
                                out=u_sb[:, q * QT * P:(q + 1) * QT * P],
                                in_=up,
                                func=mybir.ActivationFunctionType.Tanh,
                                bias=b_sb, scale=1.0,
                            )
                        ops.append(tanh_op)
                    return ops

                compute1[idx] = make_c1()

                def make_c1b(r=r, q=q, u_sb=u_sb, rs_=rs_,
                             split=(idx == B_SH * NQ - 1)):
                    ops = []
                    sp = s_psum.tile([P, QT], F32, tag="s")

                    def mk_st(j):
                        def f():
                            t_idx = q * QT + j
                            nc.tensor.matmul(
                                sp[:, j:j + 1],
                                u_sb[:, t_idx * P:(t_idx + 1) * P],
                                us_bf, start=True, stop=True,
                            )
                        return f

                    etq = et_pool.tile([P, QT], BF16, tag="et")
                    rs_[f"et{q}"] = etq

                    def exp_half(h, col):
                        def f():
                            nc.scalar.activation(
                                out=etq[:, 2 * h:2 * h + 2],
                                in_=sp[:, 2 * h:2 * h + 2],
                                func=mybir.ActivationFunctionType.Exp,
                                accum_out=rs_["rs"][:, col:col + 1],
                            )
                        return f

                    if split:
                        # halves: scores+exp for t-tiles 0-1 run while the
                        # second half's tanh is still in flight; the two
                        # exp row-sums land in rs columns q and NQ
                        ops += [mk_st(0), mk_st(1), exp_half(0, q)]
                        ops += [mk_st(2), mk_st(3), exp_half(1, NQ)]
                    else:
                        for j in range(QT):
                            ops.append(mk_st(j))

                        def exp_op():
                            nc.scalar.activation(
                                out=etq, in_=sp,
                                func=mybir.ActivationFunctionType.Exp,
                                accum_out=rs_["rs"][:, q:q + 1],
                            )
                        ops.append(exp_op)
                    return ops

                compute1b[idx] = make_c1b()

                def make_c2(r=r, q=q, x_bf=x_bf, rs_=rs_):
                    # each list entry emits one t-tile's pair of column-group
                    # matmuls back-to-back so they co-run on the PE
                    ops = []

                    def mk_p2(j):
                        def f():
                            t_idx = q * QT + j
                            for g in range(2):
                                op_t = rs_["opa"] if g == 0 else rs_["opb"]
                                kwargs = {}
                                if g == 1:
                                    kwargs["tile_position"] = (0, 64)
                                nc.tensor.matmul(
                                    op_t[64 * g:64 * g + 1, :],
                                    rs_[f"et{q}"][:, j:j + 1],
                                    x_bf[:, t_idx, g * 512:(g + 1) * 512],
                                    start=(q == 0 and j == 0),
                                    stop=(q == NQ - 1 and j == QT - 1),
                                    **kwargs,
                                )
                        return f
                    for j in range(QT):
                        ops.append(mk_p2(j))

                    if q == NQ - 1:
                        def finish(w=NQ + 1 if r == B_SH - 1 else NQ):
                            # the last row's split exp leaves two partial row
                            # sums (cols q and NQ); other rows use NQ cols
                            dnp = s_psum.tile([1, w], F32, tag="s", name="dnp")
                            nc.tensor.matmul(dnp, ones_f32, rs_["rs"][:, 0:w],
                                             start=True, stop=True)
                            dsum = out_pool.tile([1, 1], F32, tag="dsum")
                            nc.vector.reduce_sum(out=dsum, in_=dnp,
                                                 axis=mybir.AxisListType.X)
                            inv = out_pool.tile([1, 1], F32, tag="inv")
                            nc.vector.reciprocal(out=inv, in_=dsum)
                            o_sb = rs_["o_sb"]
                            nc.vector.tensor_scalar_mul(
                                o_sb[:, 0:512], rs_["opa"][0:1, :], inv)
                            nc.vector.tensor_scalar_mul(
                                o_sb[:, 512:1024], rs_["opb"][64:65, :], inv)
                            nc.scalar.dma_start(out=y[r:r + 1, :], in_=o_sb)
                        ops.append(finish)
                    return ops

                compute2[idx] = make_c2()

            # tail: c2 of the second-to-last quarter first (ready work while
            # the last copies land), then the last quarter as two interleaved
            # half-pipelines: xWa/tanh_a/scores01/exp_a/c2(j0,j1) complete
            # while xWb is still streaming
            last = B_SH * NQ - 1
            for f in compute2.pop(last - 1, []):
                f()
            c1t = compute1.pop(last, [])
            c1bt = compute1b.pop(last, [])
            c2t = compute2.pop(last, [])
            drain(c1t, 9)        # xWa x8 + tanh_a
            drain(c1bt, 3)       # scores 0,1 + exp_a
            drain(c2t, 2)        # c2 tiles 0,1
            drain(c1t, len(c1t))     # xWb x8 + tanh_b
            drain(c1bt, len(c1bt))   # scores 2,3 + exp_b
            drain(c2t, len(c2t))     # c2 tiles 2,3 + finish
            for idx in sorted(set(compute1) | set(compute1b) | set(compute2)):
                for f in compute1.pop(idx, []):
                    f()
                for f in compute1b.pop(idx, []):
                    f()
                for f in compute2.pop(idx, []):
                    f()

    nc.compile()
    return nc


_NC_CACHE = []


def _numpy_reference(x, W, b, us, mask):
    m = mask.astype(x.dtype)
    u = np.tanh(np.einsum('btd,dm->btm', x, W) + b)
    utu = np.einsum('btm,mo->bto', u, us)[..., 0]
    e = np.exp(utu - utu.max(axis=-1, keepdims=True))
    e = m * e
    a = e / e.sum(axis=-1, keepdims=True)
    return np.einsum('bt,btd->bd', a, x).astype(np.float32)


def make_in_maps(x, W, b, us):
    """Per-core input dicts; W goes in host-rearranged lhsT layout."""
    x = np.ascontiguousarray(np.asarray(x, dtype=np.float32))
    W = np.ascontiguousarray(np.asarray(W, dtype=np.float32))
    b = np.ascontiguousarray(np.asarray(b, dtype=np.float32))
    us = np.ascontiguousarray(np.asarray(us, dtype=np.float32))
    W_r = np.ascontiguousarray(W.reshape(NCD, P, M).transpose(1, 0, 2))
    b_r = np.ascontiguousarray(b.reshape(1, M))
    us_r = np.ascontiguousarray(us.reshape(M, 1).T)
    return [{
        "x": np.ascontiguousarray(x[i * B_SH:(i + 1) * B_SH]),
        "W": W_r, "b": b_r, "us": us_r,
    } for i in range(NCORES)]


def kernel(x, W, b, us, mask):
    x = np.ascontiguousarray(np.asarray(x, dtype=np.float32))
    W = np.ascontiguousarray(np.asarray(W, dtype=np.float32))
    b = np.ascontiguousarray(np.asarray(b, dtype=np.float32))
    us = np.ascontiguousarray(np.asarray(us, dtype=np.float32))
    mask = np.asarray(mask)

    if not bool(mask.all()):
        # spec guarantees an all-ones mask; fall back to exact numpy
        # reference if that ever changes
        return _numpy_reference(x, W, b, us, mask)

    if not _NC_CACHE:
        _NC_CACHE.append(_build_nc())
    nc = _NC_CACHE[0]

    in_maps = make_in_maps(x, W, b, us)
    res = run_bass_kernel_spmd(nc, in_maps, core_ids=list(range(NCORES)),
                               trace=False)
    return np.concatenate([res.results[i]["y"] for i in range(NCORES)], axis=0)
